# revision 9
# baseline (speedup 1.0000x reference)
"""Cross-attention with post-softmax multiplicative mask on 8 TRN2 NeuronCores.

Full-input contract: kernel(queries, keys, values, mask) -> (B, L, H*D) float32.

Sharding: B=4 batches x 16 heads -> 8 cores; core c handles batch c//2,
heads 8*(c%2) .. 8*(c%2)+7 (a (L, 512) column slab of q/k/v). mask (L, S)
is replicated. No collectives needed.

Per-core algorithm (8 heads = 4 head-pairs, L=S=2048, E=D=64):
  - everything runs in the "scores-transposed" layout [s, l] so that the
    A@V contraction (over s) needs no on-chip transpose of the big L x S
    attention matrix:
      scoresT tile = K2T.T @ Q2T     (lhsT=K^T [e,s], rhs=Q^T [e,l], bf16,
                                      two heads packed in array rows 0-63 /
                                      64-127)
      E^T  = exp(scale * scoresT)    (ScalarE, PSUM->SBUF, bf16 out)
      M^T  = E^T * mask^T            (VectorE, bf16 2x mode)
      outT += V.T @ M^T              (lhsT=V [s,d] natural layout)
      Z    += ones.T @ E^T           (col-tiled at tile_position (0,64);
                                      runs in array cols 64-95 concurrently
                                      with A@V in cols 0-63)
  - normalization out = outT / Z happens after a small [96,128]->[128,96]
    DMA-transpose per 128-row chunk, as a per-partition tensor_scalar.
"""

import os
import sys
from contextlib import ExitStack

import numpy as np

os.environ.setdefault("MYCRO_LOCAL_CACHE", "1")

for _p in ("/opt/trn_rl_repo",):
    if _p not in sys.path and os.path.isdir(_p):
        sys.path.insert(0, _p)

import concourse.bass as bass  # noqa: E402
import concourse.tile as tile  # noqa: E402
from concourse import bacc, bass_utils, mybir  # noqa: E402

F32 = mybir.dt.float32
BF16 = mybir.dt.bfloat16

B, L, S, DMODEL = 4, 2048, 2048, 1024
H, E = 16, 64
N_CORES = 8
DM = 512          # d_model columns per core (8 heads)
NHP = 4           # head-pairs per core
NST = S // 128    # 16 s-tiles
NLB = 2           # l blocks
LBS = L // NLB    # 1024
SCALE = 1.0 / 8.0  # 1/sqrt(E)

DIVIDE_ON_DVE = False  # tensor_scalar(op0=divide) fails walrus ISA check; use reciprocal+mul
DEBUG_DUMPS = False  # add intermediate-tensor ExternalOutputs (hp=0) for HW bisection


def build_program(repeat=1):
    """repeat>1 duplicates the whole compute body (for slope timing)."""
    nc = bacc.Bacc(
        "TRN2", target_bir_lowering=False, debug=False, num_devices=N_CORES
    )
    q = nc.dram_tensor("q", (L, DM), F32, kind="ExternalInput").ap()
    k = nc.dram_tensor("k", (L, DM), F32, kind="ExternalInput").ap()
    v = nc.dram_tensor("v", (L, DM), F32, kind="ExternalInput").ap()
    msk = nc.dram_tensor("mask", (L, S), F32, kind="ExternalInput").ap()
    out = nc.dram_tensor("out", (L, DM), F32, kind="ExternalOutput").ap()
    dbg = {}
    if DEBUG_DUMPS:
        for nm, shape, dt in [
            ("d_q2t", (128, L), BF16), ("d_k2t", (128, L), BF16),
            ("d_maskt0", (128, L), BF16), ("d_sc0", (128, LBS), F32),
            ("d_e0", (128, LBS), BF16), ("d_m0", (128, LBS), BF16),
            ("d_osb0", (96, LBS), BF16), ("d_ot0", (128, 96), BF16),
        ]:
            dbg[nm] = nc.dram_tensor(nm, shape, dt, kind="ExternalOutput").ap()

    Exp = mybir.ActivationFunctionType.Exp
    div = mybir.AluOpType.divide
    mult = mybir.AluOpType.mult

    with ExitStack() as ctx:
        tc = ctx.enter_context(tile.TileContext(nc))

        dram = ctx.enter_context(tc.tile_pool(name="dram", bufs=1, space="DRAM"))
        const_p = ctx.enter_context(tc.tile_pool(name="const", bufs=1))
        maskt_p = ctx.enter_context(tc.tile_pool(name="maskt", bufs=1))
        qk_p = ctx.enter_context(tc.tile_pool(name="qk", bufs=2))
        v_p = ctx.enter_context(tc.tile_pool(name="vsb", bufs=2))
        e_p = ctx.enter_context(tc.tile_pool(name="esb", bufs=2))
        m_p = ctx.enter_context(tc.tile_pool(name="msb", bufs=2))
        osb_p = ctx.enter_context(tc.tile_pool(name="osb", bufs=2))
        ot_p = ctx.enter_context(tc.tile_pool(name="ot", bufs=4))
        stage_p = ctx.enter_context(tc.tile_pool(name="stage", bufs=2))
        sc_p = ctx.enter_context(tc.tile_pool(name="sc", bufs=1, space="PSUM"))
        outt_p = ctx.enter_context(tc.tile_pool(name="outt", bufs=1, space="PSUM"))

        # --- prologue: bf16 copies of q/k/mask in DRAM (SWDGE casts) ---
        qbf = dram.tile([L, DM], BF16, tag="qbf")
        kbf = dram.tile([L, DM], BF16, tag="kbf")
        mbf = dram.tile([L, S], BF16, tag="mbf")
        nc.gpsimd.dma_start(qbf[:], q[:])
        nc.gpsimd.dma_start(kbf[:], k[:])

        ones = const_p.tile([128, 32], BF16, tag="ones")
        nc.vector.memset(ones[:], 1.0)

        # mask^T tiles, resident in SBUF for the whole kernel.
        # Chunked cast (f32->bf16 DRAM->DRAM) then xbar transpose per s-tile
        # so the first tiles become available quickly.
        maskt = []
        for st in range(NST):
            nc.gpsimd.dma_start(
                mbf[:, st * 128 : (st + 1) * 128], msk[:, st * 128 : (st + 1) * 128]
            )
            mt = maskt_p.tile([128, L], BF16, tag=f"maskt{st}", name=f"maskt{st}")
            nc.sync.dma_start(
                mt[:], mbf[:, st * 128 : (st + 1) * 128], transpose=True
            )
            maskt.append(mt)
            if DEBUG_DUMPS and st == 0:
                nc.sync.dma_start(dbg["d_maskt0"][:], mt[:])

        for hp in range(NHP):
            c0 = hp * 128  # first d_model column of this head pair

            q2t = qk_p.tile([128, L], BF16, tag="q2t")
            nc.sync.dma_start(q2t[:], qbf[:, c0 : c0 + 128], transpose=True)
            k2t = qk_p.tile([128, L], BF16, tag="k2t")
            nc.sync.dma_start(k2t[:], kbf[:, c0 : c0 + 128], transpose=True)
            if DEBUG_DUMPS and hp == 0:
                nc.sync.dma_start(dbg["d_q2t"][:], q2t[:])
                nc.sync.dma_start(dbg["d_k2t"][:], k2t[:])

            # V in natural [s, d] layout, folded to [128, 16, 128] bf16
            vsb = v_p.tile([128, NST, 128], BF16, tag="vsb")
            v_view = v[:, c0 : c0 + 128].rearrange("(c p) d -> p c d", p=128)
            nc.gpsimd.dma_start(vsb[:], v_view)

            out_stage = stage_p.tile([128, L // 128, 128], F32, tag="out_stage")

            for lb in range(NLB):
                lc0 = lb * LBS

                outt = []
                for h in range(2):
                    outt.append(outt_p.tile([96, LBS], F32, tag=f"outt{h}", name=f"outt{h}"))

                for st in range(NST):
                    first, last = st == 0, st == NST - 1
                    sc = []
                    for h in range(2):
                        sch = sc_p.tile([128, LBS], F32, tag=f"sc{h}", name=f"sc{h}")
                        sc.append(sch)
                        for n in range(2):
                            ns = slice(n * 512, (n + 1) * 512)
                            nc.tensor.matmul(
                                sch[:, ns],
                                lhsT=k2t[h * 64 : (h + 1) * 64, st * 128 : (st + 1) * 128],
                                rhs=q2t[h * 64 : (h + 1) * 64, lc0 + n * 512 : lc0 + (n + 1) * 512],
                                start=True,
                                stop=True,
                            )
                    for h in range(2):
                        e = e_p.tile([128, LBS], BF16, tag=f"e{h}")
                        if DEBUG_DUMPS and hp == 0 and lb == 0 and st == 0 and h == 0:
                            scb = e_p.tile([128, LBS], F32, tag="scb", name="scb")
                            nc.vector.tensor_copy(scb[:], sc[h][:])
                            nc.sync.dma_start(dbg["d_sc0"][:], scb[:])
                        nc.scalar.activation(e[:], sc[h][:], Exp, scale=SCALE)
                        m = m_p.tile([128, LBS], BF16, tag=f"m{h}")
                        nc.vector.tensor_tensor(
                            m[:], e[:], maskt[st][:, lc0 : lc0 + LBS], op=mult
                        )
                        if DEBUG_DUMPS and hp == 0 and lb == 0 and st == 0 and h == 0:
                            nc.sync.dma_start(dbg["d_e0"][:], e[:])
                            nc.sync.dma_start(dbg["d_m0"][:], m[:])
                        for n in range(2):
                            ns = slice(n * 512, (n + 1) * 512)
                            nc.tensor.matmul(
                                outt[h][0:64, ns],
                                lhsT=vsb[:, st, h * 64 : (h + 1) * 64],
                                rhs=m[:, ns],
                                start=first,
                                stop=last,
                                tile_position=(0, 0),
                                skip_group_check=True,
                            )
                            nc.tensor.matmul(
                                outt[h][64:96, ns],
                                lhsT=ones[:],
                                rhs=e[:, ns],
                                start=first,
                                stop=last,
                                tile_position=(0, 64),
                                skip_group_check=True,
                            )

                # evacuate, transpose back to [l, d], normalize by Z
                for h in range(2):
                    osb = osb_p.tile([96, LBS], BF16, tag=f"osb{h}")
                    nc.vector.tensor_copy(osb[:], outt[h][:])
                    if DEBUG_DUMPS and hp == 0 and lb == 0 and h == 0:
                        nc.sync.dma_start(dbg["d_osb0"][:], osb[:])
                    for lc in range(LBS // 128):
                        ot = ot_p.tile([128, 96], BF16, tag=f"ot{h}")
                        nc.sync.dma_start(
                            ot[:], osb[:, lc * 128 : (lc + 1) * 128], transpose=True
                        )
                        if DEBUG_DUMPS and hp == 0 and lb == 0 and h == 0 and lc == 0:
                            nc.sync.dma_start(dbg["d_ot0"][:], ot[:])
                        dst = out_stage[:, lb * (LBS // 128) + lc, h * 64 : (h + 1) * 64]
                        zf = ot_p.tile([128, 1], F32, tag=f"zf{h}")
                        nc.vector.tensor_copy(zf[:], ot[:, 64:65])
                        if DIVIDE_ON_DVE:
                            nc.vector.tensor_scalar(
                                dst, ot[:, 0:64], zf[:], None, op0=div
                            )
                        else:
                            rz = ot_p.tile([128, 1], F32, tag=f"rz{h}")
                            nc.vector.reciprocal(rz[:], zf[:])
                            nc.vector.tensor_scalar(
                                dst, ot[:, 0:64], rz[:], None, op0=mult
                            )

            out_view = out[:, c0 : c0 + 128].rearrange("(c p) d -> p c d", p=128)
            nc.sync.dma_start(out_view, out_stage[:])

    nc.compile()
    return nc


_CACHE = {}


def _get_nc():
    if "nc" not in _CACHE:
        _CACHE["nc"] = build_program()
    return _CACHE["nc"]


def _shard(queries, keys, values, mask):
    mask32 = np.ascontiguousarray(mask, dtype=np.float32)
    in_maps = []
    for c in range(N_CORES):
        b, hb = c // 2, c % 2
        cols = slice(hb * DM, (hb + 1) * DM)
        in_maps.append(
            {
                "q": np.ascontiguousarray(queries[b, :, cols], dtype=np.float32),
                "k": np.ascontiguousarray(keys[b, :, cols], dtype=np.float32),
                "v": np.ascontiguousarray(values[b, :, cols], dtype=np.float32),
                "mask": mask32,
            }
        )
    return in_maps


def _gather(results):
    # The reference's output layout is (B,H,L,D) reshaped contiguously to
    # (B, L, H*D) ("mix"): head h occupies output rows [h*128, (h+1)*128).
    # Core (b, hb) computes heads hb*8..hb*8+7 in standard [l, h_local*64+e]
    # layout -> its block is rows [hb*1024, (hb+1)*1024) after the remap.
    out = np.empty((B, L, DMODEL), np.float32)
    for c in range(N_CORES):
        b, hb = c // 2, c % 2
        core = results[c]["out"]  # (L, 512) = [l, h_local*64+e]
        block = (
            core.reshape(L, DM // E, E).transpose(1, 0, 2).reshape(L * DM // DMODEL, DMODEL)
        )
        out[b, hb * (L // 2) : (hb + 1) * (L // 2), :] = block
    return out


def run(queries, keys, values, mask, trace=False):
    """Run on hardware; returns (output, BassKernelResults)."""
    nc = _get_nc()
    in_maps = _shard(queries, keys, values, mask)
    res = bass_utils.run_bass_kernel_spmd(
        nc, in_maps, core_ids=list(range(N_CORES)), trace=trace
    )
    return _gather(res.results), res


def run_timed(queries, keys, values, mask, iters=5):
    """Time the NEFF execution with device-resident inputs.

    Mirrors bass2jax.run_bass_via_pjrt's multi-core path, but device_puts
    the sharded inputs first so repeated calls measure dispatch + execute
    without host transfers. Returns (output, best_seconds_per_call).
    """
    import time

    import jax
    from jax.sharding import Mesh, NamedSharding, PartitionSpec
    from jax.experimental.shard_map import shard_map
    from concourse import bass2jax
    from concourse.bass2jax import _bass_exec_p, partition_id_tensor

    bass2jax.install_neuronx_cc_hook()
    nc = _get_nc()
    in_maps = _shard(queries, keys, values, mask)

    partition_name = nc.partition_id_tensor.name if nc.partition_id_tensor else None
    in_names, out_names, out_avals, zero_outs = [], [], [], []
    for alloc in nc.m.functions[0].allocations:
        if not isinstance(alloc, mybir.MemoryLocationSet):
            continue
        name = alloc.memorylocations[0].name
        if alloc.kind == "ExternalInput":
            if name != partition_name:
                in_names.append(name)
        elif alloc.kind == "ExternalOutput":
            shape = tuple(alloc.tensor_shape)
            dtype = mybir.dt.np(alloc.dtype)
            out_names.append(name)
            out_avals.append(jax.core.ShapedArray(shape, dtype))
            zero_outs.append(np.zeros(shape, dtype))
    n_params = len(in_names)
    n_outs = len(out_avals)
    all_names = in_names + out_names
    if partition_name is not None:
        all_names = all_names + [partition_name]

    def _body(*args):
        operands = list(args)
        if partition_name is not None:
            operands.append(partition_id_tensor())
        return tuple(
            _bass_exec_p.bind(
                *operands,
                out_avals=tuple(out_avals),
                in_names=tuple(all_names),
                out_names=tuple(out_names),
                lowering_input_output_aliases=(),
                sim_require_finite=True,
                sim_require_nnan=True,
                nc=nc,
            )
        )

    devices = jax.devices()[:N_CORES]
    mesh = Mesh(np.asarray(devices), ("core",))
    spec = NamedSharding(mesh, PartitionSpec("core"))
    donate = tuple(range(n_params, n_params + n_outs))
    sharded = jax.jit(
        shard_map(
            _body,
            mesh=mesh,
            in_specs=(PartitionSpec("core"),) * (n_params + n_outs),
            out_specs=(PartitionSpec("core"),) * n_outs,
            check_rep=False,
        ),
        donate_argnums=donate,
        keep_unused=True,
    )
    concat_in = [
        jax.device_put(
            np.concatenate([in_maps[c][nm] for c in range(N_CORES)], axis=0), spec
        )
        for nm in in_names
    ]
    concat_zeros_np = [
        np.zeros((N_CORES * z.shape[0], *z.shape[1:]), z.dtype) for z in zero_outs
    ]

    best = None
    out_arrs = None
    for _ in range(iters):
        zeros_dev = [jax.device_put(z, spec) for z in concat_zeros_np]
        for z in zeros_dev:
            z.block_until_ready()
        t0 = time.perf_counter()
        out_arrs = sharded(*concat_in, *zeros_dev)
        for o in out_arrs:
            o.block_until_ready()
        dt = time.perf_counter() - t0
        if best is None or dt < best:
            best = dt

    results = [
        {
            nm: np.asarray(out_arrs[i]).reshape(N_CORES, *out_avals[i].shape)[c]
            for i, nm in enumerate(out_names)
        }
        for c in range(N_CORES)
    ]
    return _gather(results), best


def kernel(queries, keys, values, mask):
    out, _ = run(queries, keys, values, mask)
    return out
